# revision 3
# baseline (speedup 1.0000x reference)
"""FBSNN net_u_Du kernel for 8 trn2 NeuronCores.

Computes, for u(s) = W2 @ sin(W1 @ s + b1) + b2 with s = [t, x]:
  u            (M,1)
  DuDx = g[:,1:], DuDt = g[:,:1]  with  g = (W2 o cos Z) @ W1
  D2uDx2[m]    = V^T diag(-W2 o sin z_m) V,  V = W1[:,1:]

Key reduction: the per-sample Hessians batch into one dense matmul
  D2[m, j*100+k] = sum_h S[h,m] * P[h, j*100+k]
with S[h,m] = W2[h]*sin(Z[m,h]) and P[h,jk] = -V[h,j]*V[h,k].
Data parallel over M=4096 paths -> 512 per core; weights replicated.
"""

import numpy as np

import concourse.bacc as bacc
import concourse.mybir as mybir
import concourse.tile as tile
from concourse.bass_utils import run_bass_kernel_spmd

N_CORES = 8
M_FULL = 4096
MC = M_FULL // N_CORES  # 512 paths per core
D = 100
DP1 = D + 1  # 101
H = 256  # hidden width
F32 = mybir.dt.float32

NCHUNK = 500  # matmul free-dim per Hessian matmul (<=512 fp32 PSUM bank)
CPG = 4  # chunks per DMA group
GCOLS = NCHUNK * CPG  # 2000
NGROUPS = (D * D) // GCOLS  # 5
NM = MC // 128  # 4 m-chunks of 128 paths

# set by test harness to profile; kernel() records exec time here
TRACE = False
LAST_EXEC_NS = None

_CACHE = {}


def _build():
    nc = bacc.Bacc(None, target_bir_lowering=False, debug=False)
    sin_f = mybir.ActivationFunctionType.Sin
    copy_f = mybir.ActivationFunctionType.Copy
    mult = mybir.AluOpType.mult

    xt_d = nc.dram_tensor("XT", [DP1, MC], F32, kind="ExternalInput")
    w1t_d = nc.dram_tensor("W1T", [DP1, H], F32, kind="ExternalInput")
    w1_d = nc.dram_tensor("W1", [H, DP1], F32, kind="ExternalInput")
    negv_d = nc.dram_tensor("negV", [H, D], F32, kind="ExternalInput")
    b1c_d = nc.dram_tensor("b1c", [H, 1], F32, kind="ExternalInput")
    b1p_d = nc.dram_tensor("b1p", [H, 1], F32, kind="ExternalInput")
    w2c_d = nc.dram_tensor("W2c", [H, 1], F32, kind="ExternalInput")
    b2c_d = nc.dram_tensor("b2c", [1, 1], F32, kind="ExternalInput")

    u_d = nc.dram_tensor("u", [1, MC], F32, kind="ExternalOutput")
    dudt_d = nc.dram_tensor("DuDt", [MC, 1], F32, kind="ExternalOutput")
    dudx_d = nc.dram_tensor("DuDx", [MC, D], F32, kind="ExternalOutput")
    d2_d = nc.dram_tensor("D2", [MC, D * D], F32, kind="ExternalOutput")

    with tile.TileContext(nc) as tc:
        with (
            tc.tile_pool(name="const", bufs=1) as const,
            tc.tile_pool(name="work", bufs=2) as work,
            tc.tile_pool(name="pP", bufs=3) as pP,
            tc.tile_pool(name="stage", bufs=4) as stage_p,
            tc.tile_pool(name="psZ", bufs=2, space="PSUM") as psZ,
            tc.tile_pool(name="psH", bufs=4, space="PSUM") as psH,
            tc.tile_pool(name="psU", bufs=1, space="PSUM") as psU,
            tc.tile_pool(name="psG", bufs=1, space="PSUM") as psG,
        ):
            # ---- load constants / inputs ----
            xt = const.tile([DP1, MC], F32)
            nc.sync.dma_start(xt[:], xt_d[:])
            w1t = const.tile([DP1, H], F32)
            nc.sync.dma_start(w1t[:], w1t_d[:])
            w1 = [const.tile([128, DP1], F32, tag=f"w1_{k}", name=f"w1_{k}") for k in range(2)]
            negv = [const.tile([128, D], F32, tag=f"negv_{k}", name=f"negv_{k}") for k in range(2)]
            b1c = [const.tile([128, 1], F32, tag=f"b1c_{k}", name=f"b1c_{k}") for k in range(2)]
            b1p = [const.tile([128, 1], F32, tag=f"b1p_{k}", name=f"b1p_{k}") for k in range(2)]
            w2c = [const.tile([128, 1], F32, tag=f"w2c_{k}", name=f"w2c_{k}") for k in range(2)]
            for k in range(2):
                hs = slice(k * 128, (k + 1) * 128)
                nc.sync.dma_start(w1[k][:], w1_d[hs, :])
                nc.sync.dma_start(negv[k][:], negv_d[hs, :])
                nc.sync.dma_start(b1c[k][:], b1c_d[hs, :])
                nc.sync.dma_start(b1p[k][:], b1p_d[hs, :])
                nc.sync.dma_start(w2c[k][:], w2c_d[hs, :])
            b2t = const.tile([1, 1], F32)
            nc.sync.dma_start(b2t[:], b2c_d[:])
            ones = const.tile([128, 1], F32)
            nc.vector.memset(ones[:], 1.0)

            # ---- trig phase: Z^T = W1 @ [t,X]^T; S = W2*sin, C = W2*cos ----
            S = [const.tile([128, MC], F32, tag=f"S_{k}", name=f"S_{k}") for k in range(2)]
            C = [const.tile([128, MC], F32, tag=f"C_{k}", name=f"C_{k}") for k in range(2)]
            for k in range(2):
                ztp = psZ.tile([128, MC], F32, tag="zt")
                nc.tensor.matmul(
                    ztp[:], w1t[:, k * 128 : (k + 1) * 128], xt[:],
                    start=True, stop=True,
                )
                sin_t = work.tile([128, MC], F32, tag="trigtmp")
                nc.scalar.activation(sin_t[:], ztp[:], sin_f, bias=b1c[k][:])
                nc.vector.tensor_scalar_mul(S[k][:], sin_t[:], w2c[k][:])
                cos_t = work.tile([128, MC], F32, tag="trigtmp")
                nc.scalar.activation(cos_t[:], ztp[:], sin_f, bias=b1p[k][:])
                nc.vector.tensor_scalar_mul(C[k][:], cos_t[:], w2c[k][:])

            # ---- Hessian: D2[m, jk] = sum_h S[h,m] * P[h, jk] ----
            jpg = GCOLS // D  # j-blocks per group (20)
            for gi in range(NGROUPS):
                Pg = []
                for k in range(2):
                    pt = pP.tile([128, GCOLS], F32, tag=f"P_{k}", name=f"P_{k}_{gi}")
                    js = slice(gi * jpg, (gi + 1) * jpg)
                    nc.vector.tensor_tensor(
                        out=pt[:].rearrange("p (j l) -> p j l", l=D),
                        in0=negv[k][:, js, None].to_broadcast([128, jpg, D]),
                        in1=w1[k][:, None, 1:DP1].to_broadcast([128, jpg, D]),
                        op=mult,
                    )
                    Pg.append(pt)
                for m in range(NM):
                    ms = slice(m * 128, (m + 1) * 128)
                    st = stage_p.tile([128, GCOLS], F32, tag="stage")
                    pss = [
                        psH.tile([128, NCHUNK], F32, tag="hess", name=f"ps_{gi}_{m}_{c}")
                        for c in range(CPG)
                    ]
                    for k in range(2):
                        for c in range(CPG):
                            cs = slice(c * NCHUNK, (c + 1) * NCHUNK)
                            nc.tensor.matmul(
                                pss[c][:], S[k][:, ms], Pg[k][:, cs],
                                start=(k == 0), stop=(k == 1),
                            )
                    for c in range(CPG):
                        cs = slice(c * NCHUNK, (c + 1) * NCHUNK)
                        if c % 2 == 0:
                            nc.vector.tensor_copy(st[:, cs], pss[c][:])
                        else:
                            nc.scalar.copy(st[:, cs], pss[c][:])
                    nc.sync.dma_start(
                        d2_d[ms, gi * GCOLS : (gi + 1) * GCOLS], st[:]
                    )

            # ---- u = ones^T @ S + b2 ----
            up = psU.tile([1, MC], F32)
            for k in range(2):
                nc.tensor.matmul(
                    up[:], ones[:], S[k][:], start=(k == 0), stop=(k == 1)
                )
            u_sb = work.tile([1, MC], F32, tag="usb")
            nc.vector.tensor_scalar_add(u_sb[:], up[:], b2t[:])
            nc.sync.dma_start(u_d[:], u_sb[:])

            # ---- g = C^T @ W1 -> DuDt | DuDx ----
            for m in range(NM):
                ms = slice(m * 128, (m + 1) * 128)
                gp = psG.tile([128, DP1], F32, tag="g", name=f"gp_{m}")
                for k in range(2):
                    nc.tensor.matmul(
                        gp[:], C[k][:, ms], w1[k][:], start=(k == 0), stop=(k == 1)
                    )
                g_sb = work.tile([128, DP1], F32, tag="gsb")
                nc.vector.tensor_copy(g_sb[:], gp[:])
                nc.sync.dma_start(dudt_d[ms, :], g_sb[:, 0:1])
                nc.sync.dma_start(dudx_d[ms, :], g_sb[:, 1:DP1])

    nc.compile()
    return nc


def kernel(t, X, W1, b1, W2, b2):
    global LAST_EXEC_NS
    t = np.ascontiguousarray(np.asarray(t, dtype=np.float32))
    X = np.ascontiguousarray(np.asarray(X, dtype=np.float32))
    W1 = np.ascontiguousarray(np.asarray(W1, dtype=np.float32))
    b1 = np.ascontiguousarray(np.asarray(b1, dtype=np.float32))
    W2 = np.ascontiguousarray(np.asarray(W2, dtype=np.float32))
    b2 = np.ascontiguousarray(np.asarray(b2, dtype=np.float32))

    xaug_t = np.ascontiguousarray(np.concatenate([t, X], axis=1).T)  # (101, 4096)
    w1t = np.ascontiguousarray(W1.T)  # (101, 256)
    negv = np.ascontiguousarray(-W1[:, 1:])  # (256, 100)
    b1c = np.ascontiguousarray(b1.reshape(H, 1))
    b1p = np.ascontiguousarray(b1c + np.float32(np.pi / 2))
    w2c = np.ascontiguousarray(W2.reshape(H, 1))
    b2c = np.ascontiguousarray(b2.reshape(1, 1))

    if "nc" not in _CACHE:
        _CACHE["nc"] = _build()
    nc = _CACHE["nc"]

    in_maps = []
    for i in range(N_CORES):
        in_maps.append(
            {
                "XT": np.ascontiguousarray(xaug_t[:, i * MC : (i + 1) * MC]),
                "W1T": w1t,
                "W1": W1,
                "negV": negv,
                "b1c": b1c,
                "b1p": b1p,
                "W2c": w2c,
                "b2c": b2c,
            }
        )

    res = run_bass_kernel_spmd(nc, in_maps, list(range(N_CORES)), trace=TRACE)
    LAST_EXEC_NS = res.exec_time_ns

    u = np.concatenate(
        [res.results[i]["u"].reshape(MC, 1) for i in range(N_CORES)], axis=0
    )
    dudt = np.concatenate([res.results[i]["DuDt"] for i in range(N_CORES)], axis=0)
    dudx = np.concatenate([res.results[i]["DuDx"] for i in range(N_CORES)], axis=0)
    d2 = np.concatenate(
        [res.results[i]["D2"].reshape(MC, D, D) for i in range(N_CORES)], axis=0
    )
    return u, dudx, dudt, d2


# revision 8
# speedup vs baseline: 1.5909x; 1.5909x over previous
"""FBSNN net_u_Du kernel for 8 trn2 NeuronCores.

Computes, for u(s) = W2 @ sin(W1 @ s + b1) + b2 with s = [t, x]:
  u            (M,1)
  DuDx = g[:,1:], DuDt = g[:,:1]  with  g = (W2 o cos Z) @ W1
  D2uDx2[m]    = V^T diag(-W2 o sin z_m) V,  V = W1[:,1:]

Key reduction: the per-sample Hessians batch into one dense matmul
  D2[m, j*100+k] = sum_h S[h,m] * P[h, j*100+k]
with S[h,m] = W2[h]*sin(Z[m,h]) and P[h,jk] = -V[h,j]*V[h,k].
Data parallel over M=4096 paths -> 512 per core; weights replicated.
"""

import numpy as np

import concourse.bacc as bacc
import concourse.mybir as mybir
import concourse.tile as tile
from concourse.bass_utils import run_bass_kernel_spmd

N_CORES = 8
M_FULL = 4096
MC = M_FULL // N_CORES  # 512 paths per core
D = 100
DP1 = D + 1  # 101
H = 256  # hidden width
F32 = mybir.dt.float32
F16 = mybir.dt.float16

NCHUNK = 500  # matmul free-dim per Hessian matmul (<=512 fp32 PSUM bank)
CPG = 4  # chunks per DMA group
GCOLS = NCHUNK * CPG  # 2000
NGROUPS = (D * D) // GCOLS  # 5
NM = MC // 128  # 4 m-chunks of 128 paths

# set by test harness to profile; kernel() records exec time here
TRACE = False
LAST_EXEC_NS = None

_CACHE = {}


def _build():
    nc = bacc.Bacc(None, target_bir_lowering=False, debug=False)
    sin_f = mybir.ActivationFunctionType.Sin
    copy_f = mybir.ActivationFunctionType.Copy
    mult = mybir.AluOpType.mult

    xt_d = nc.dram_tensor("XT", [DP1, MC], F32, kind="ExternalInput")
    w1t_d = nc.dram_tensor("W1T", [DP1, H], F32, kind="ExternalInput")
    w1_d = nc.dram_tensor("W1", [H, DP1], F32, kind="ExternalInput")
    negv_d = nc.dram_tensor("negV", [H, D], F32, kind="ExternalInput")
    b1c_d = nc.dram_tensor("b1c", [H, 1], F32, kind="ExternalInput")
    b1p_d = nc.dram_tensor("b1p", [H, 1], F32, kind="ExternalInput")
    w2c_d = nc.dram_tensor("W2c", [H, 1], F32, kind="ExternalInput")
    b2c_d = nc.dram_tensor("b2c", [1, 1], F32, kind="ExternalInput")

    u_d = nc.dram_tensor("u", [1, MC], F32, kind="ExternalOutput")
    dudt_d = nc.dram_tensor("DuDt", [MC, 1], F32, kind="ExternalOutput")
    dudx_d = nc.dram_tensor("DuDx", [MC, D], F32, kind="ExternalOutput")
    d2_d = nc.dram_tensor("D2", [MC, D * D], F32, kind="ExternalOutput")

    with tile.TileContext(nc) as tc:
        with (
            tc.tile_pool(name="const", bufs=1) as const,
            tc.tile_pool(name="work", bufs=2) as work,
            tc.tile_pool(name="pP", bufs=3) as pP,
            tc.tile_pool(name="stage", bufs=4) as stage_p,
            tc.tile_pool(name="psZ", bufs=2, space="PSUM") as psZ,
            tc.tile_pool(name="psH", bufs=4, space="PSUM") as psH,
            tc.tile_pool(name="psU", bufs=1, space="PSUM") as psU,
            tc.tile_pool(name="psG", bufs=1, space="PSUM") as psG,
        ):
            # ---- load constants / inputs ----
            xt = const.tile([DP1, MC], F32)
            nc.sync.dma_start(xt[:], xt_d[:])
            w1t = const.tile([DP1, H], F32)
            nc.sync.dma_start(w1t[:], w1t_d[:])
            w1 = [const.tile([128, DP1], F32, tag=f"w1_{k}", name=f"w1_{k}") for k in range(2)]
            negv = [const.tile([128, D], F32, tag=f"negv_{k}", name=f"negv_{k}") for k in range(2)]
            b1c = [const.tile([128, 1], F32, tag=f"b1c_{k}", name=f"b1c_{k}") for k in range(2)]
            b1p = [const.tile([128, 1], F32, tag=f"b1p_{k}", name=f"b1p_{k}") for k in range(2)]
            w2c = [const.tile([128, 1], F32, tag=f"w2c_{k}", name=f"w2c_{k}") for k in range(2)]
            for k in range(2):
                hs = slice(k * 128, (k + 1) * 128)
                nc.sync.dma_start(w1[k][:], w1_d[hs, :])
                nc.sync.dma_start(negv[k][:], negv_d[hs, :])
                nc.sync.dma_start(b1c[k][:], b1c_d[hs, :])
                nc.sync.dma_start(b1p[k][:], b1p_d[hs, :])
                nc.sync.dma_start(w2c[k][:], w2c_d[hs, :])
            b2t = const.tile([1, 1], F32)
            nc.sync.dma_start(b2t[:], b2c_d[:])
            ones = const.tile([128, 1], F32)
            nc.vector.memset(ones[:], 1.0)

            # ---- trig phase: Z^T = W1 @ [t,X]^T; S = W2*sin, C = W2*cos ----
            # HW Sin is only accurate on [-pi, pi]: range-reduce via
            # w = y - 2pi*round(y/2pi) (f32->i32 cast rounds to nearest).
            pihalf = const.tile([128, 1], F32)
            nc.vector.memset(pihalf[:], float(np.pi / 2))
            S32 = [const.tile([128, MC], F32, tag=f"S32_{k}", name=f"S32_{k}") for k in range(2)]
            S16 = [const.tile([128, MC], F16, tag=f"S16_{k}", name=f"S16_{k}") for k in range(2)]
            C = [const.tile([128, MC], F32, tag=f"C_{k}", name=f"C_{k}") for k in range(2)]
            inv2pi = float(1.0 / (2.0 * np.pi))
            twopi = float(2.0 * np.pi)
            for k in range(2):
                ztp = psZ.tile([128, MC], F32, tag="zt")
                nc.tensor.matmul(
                    ztp[:], w1t[:, k * 128 : (k + 1) * 128], xt[:],
                    start=True, stop=True,
                )
                y = work.tile([128, MC], F32, tag="y")
                nc.vector.tensor_scalar_add(y[:], ztp[:], b1c[k][:])
                ki = work.tile([128, MC], mybir.dt.int32, tag="ki")
                nc.vector.tensor_scalar(
                    out=ki[:], in0=y[:], scalar1=inv2pi, scalar2=None,
                    op0=mybir.AluOpType.mult,
                )
                kf = work.tile([128, MC], F32, tag="kf")
                nc.vector.tensor_scalar(
                    out=kf[:], in0=ki[:], scalar1=twopi, scalar2=None,
                    op0=mybir.AluOpType.mult,
                )
                w = work.tile([128, MC], F32, tag="wred")
                nc.vector.tensor_tensor(
                    out=w[:], in0=y[:], in1=kf[:], op=mybir.AluOpType.subtract
                )
                # sin path
                sin_t = work.tile([128, MC], F32, tag="trigtmp")
                nc.scalar.activation(sin_t[:], w[:], sin_f)
                nc.vector.tensor_scalar_mul(S32[k][:], sin_t[:], w2c[k][:])
                nc.vector.tensor_copy(S16[k][:], S32[k][:])
                # cos path: cos(z) = sin(w + pi/2), wrapped by one period if
                # w > pi/2 so the Sin argument stays within [-pi, pi].
                hi = work.tile([128, MC], F32, tag="hi")
                nc.vector.tensor_scalar(
                    out=hi[:], in0=w[:], scalar1=float(np.pi / 2), scalar2=-twopi,
                    op0=mybir.AluOpType.is_gt, op1=mybir.AluOpType.mult,
                )
                wc = work.tile([128, MC], F32, tag="wc")
                nc.vector.tensor_tensor(
                    out=wc[:], in0=w[:], in1=hi[:], op=mybir.AluOpType.add
                )
                cos_t = work.tile([128, MC], F32, tag="trigtmp")
                nc.scalar.activation(cos_t[:], wc[:], sin_f, bias=pihalf[:])
                nc.vector.tensor_scalar_mul(C[k][:], cos_t[:], w2c[k][:])

            # ---- Hessian: D2[m, jk] = sum_h S[h,m] * P[h, jk] ----
            # fp16 operands: fp32 matmul costs 2x on the PE (two-pass) and
            # its LDWEIGHTS can't fast-load; fp16 streams at full rate and
            # accumulates in fp32 PSUM.
            negv16 = [const.tile([128, D], F16, tag=f"negv16_{k}", name=f"negv16_{k}") for k in range(2)]
            w1v16 = [const.tile([128, D], F16, tag=f"w1v16_{k}", name=f"w1v16_{k}") for k in range(2)]
            for k in range(2):
                nc.vector.tensor_copy(negv16[k][:], negv[k][:])
                nc.vector.tensor_copy(w1v16[k][:], w1[k][:, 1:DP1])
            jpg = GCOLS // D  # j-blocks per group (20)
            for gi in range(NGROUPS):
                Pg = []
                for k in range(2):
                    pt = pP.tile([128, GCOLS], F16, tag=f"P_{k}", name=f"P_{k}_{gi}")
                    js = slice(gi * jpg, (gi + 1) * jpg)
                    nc.vector.tensor_tensor(
                        out=pt[:].rearrange("p (j l) -> p j l", l=D),
                        in0=negv16[k][:, js, None].to_broadcast([128, jpg, D]),
                        in1=w1v16[k][:, None, :].to_broadcast([128, jpg, D]),
                        op=mult,
                    )
                    Pg.append(pt)
                for m in range(NM):
                    ms = slice(m * 128, (m + 1) * 128)
                    st = stage_p.tile([128, GCOLS], F32, tag="stage")
                    pss = [
                        psH.tile([128, NCHUNK], F32, tag="hess", name=f"ps_{gi}_{m}_{c}")
                        for c in range(CPG)
                    ]
                    for k in range(2):
                        for c in range(CPG):
                            cs = slice(c * NCHUNK, (c + 1) * NCHUNK)
                            nc.tensor.matmul(
                                pss[c][:], S16[k][:, ms], Pg[k][:, cs],
                                start=(k == 0), stop=(k == 1),
                            )
                    for c in range(CPG):
                        cs = slice(c * NCHUNK, (c + 1) * NCHUNK)
                        if c % 2 == 0:
                            nc.vector.tensor_copy(st[:, cs], pss[c][:])
                        else:
                            nc.scalar.copy(st[:, cs], pss[c][:])
                    nc.sync.dma_start(
                        d2_d[ms, gi * GCOLS : (gi + 1) * GCOLS], st[:]
                    )

            # ---- u = ones^T @ S + b2 ----
            up = psU.tile([1, MC], F32)
            for k in range(2):
                nc.tensor.matmul(
                    up[:], ones[:], S32[k][:], start=(k == 0), stop=(k == 1)
                )
            u_sb = work.tile([1, MC], F32, tag="usb")
            nc.vector.tensor_scalar_add(u_sb[:], up[:], b2t[:])
            nc.sync.dma_start(u_d[:], u_sb[:])

            # ---- g = C^T @ W1 -> DuDt | DuDx ----
            for m in range(NM):
                ms = slice(m * 128, (m + 1) * 128)
                gp = psG.tile([128, DP1], F32, tag="g", name=f"gp_{m}")
                for k in range(2):
                    nc.tensor.matmul(
                        gp[:], C[k][:, ms], w1[k][:], start=(k == 0), stop=(k == 1)
                    )
                g_sb = work.tile([128, DP1], F32, tag="gsb")
                nc.vector.tensor_copy(g_sb[:], gp[:])
                nc.sync.dma_start(dudt_d[ms, :], g_sb[:, 0:1])
                nc.sync.dma_start(dudx_d[ms, :], g_sb[:, 1:DP1])

    nc.compile()
    return nc


def kernel(t, X, W1, b1, W2, b2):
    global LAST_EXEC_NS
    t = np.ascontiguousarray(np.asarray(t, dtype=np.float32))
    X = np.ascontiguousarray(np.asarray(X, dtype=np.float32))
    W1 = np.ascontiguousarray(np.asarray(W1, dtype=np.float32))
    b1 = np.ascontiguousarray(np.asarray(b1, dtype=np.float32))
    W2 = np.ascontiguousarray(np.asarray(W2, dtype=np.float32))
    b2 = np.ascontiguousarray(np.asarray(b2, dtype=np.float32))

    xaug_t = np.ascontiguousarray(np.concatenate([t, X], axis=1).T)  # (101, 4096)
    w1t = np.ascontiguousarray(W1.T)  # (101, 256)
    negv = np.ascontiguousarray(-W1[:, 1:])  # (256, 100)
    b1c = np.ascontiguousarray(b1.reshape(H, 1))
    b1p = np.ascontiguousarray(b1c + np.float32(np.pi / 2))
    w2c = np.ascontiguousarray(W2.reshape(H, 1))
    b2c = np.ascontiguousarray(b2.reshape(1, 1))

    if "nc" not in _CACHE:
        _CACHE["nc"] = _build()
    nc = _CACHE["nc"]

    in_maps = []
    for i in range(N_CORES):
        in_maps.append(
            {
                "XT": np.ascontiguousarray(xaug_t[:, i * MC : (i + 1) * MC]),
                "W1T": w1t,
                "W1": W1,
                "negV": negv,
                "b1c": b1c,
                "b1p": b1p,
                "W2c": w2c,
                "b2c": b2c,
            }
        )

    res = run_bass_kernel_spmd(nc, in_maps, list(range(N_CORES)), trace=TRACE)
    LAST_EXEC_NS = res.exec_time_ns

    u = np.concatenate(
        [res.results[i]["u"].reshape(MC, 1) for i in range(N_CORES)], axis=0
    )
    dudt = np.concatenate([res.results[i]["DuDt"] for i in range(N_CORES)], axis=0)
    dudx = np.concatenate([res.results[i]["DuDx"] for i in range(N_CORES)], axis=0)
    d2 = np.concatenate(
        [res.results[i]["D2"].reshape(MC, D, D) for i in range(N_CORES)], axis=0
    )
    return u, dudx, dudt, d2
